# revision 1
# baseline (speedup 1.0000x reference)
"""ExpLog Dice loss kernel for Trainium2 (8 NeuronCores, SPMD data-parallel).

Math
----
reference computes, for cls_score [N, C] and integer labels [N]:
    log_probs = log_softmax(cls_score, axis=1)
    ni_c  = logsumexp_{n: label==c} log_probs[n, c]
    npr_c = logsumexp_n           log_probs[n, c]
    counts_c = #{n: label==c}
    ... tiny C-length final loss.

Since cls_score ~ N(0,1), exp(x) never overflows fp32, so logsumexps become
plain sums of probabilities:
    S_c = sum_n exp(x[n,c]) / D_n        (npr_c = log S_c)
    T_c = sum_{n:label=c} exp(x[n,c])/D_n (ni_c = log T_c)
    D_n = sum_c exp(x[n,c])

Device strategy (per core, N/8 = 131072 points):
  - layout: [128 partitions x S pages x 32 classes], one point per page
  - ACT: e = exp(x)                               (1 pass)
  - DVE: D = reduce_add over class axis           (1 pass)
  - DVE: r = 1/D
  - DVE custom op ONEHOT_GATHER: m[p,s,c] = (c == label[p,s]) ? e : 0 (1 pass)
  - PE:  with lhsT = r (per-page reciprocal columns), rhs = e resp. m,
         accumulate [GM, GM*C] PSUM blocks whose diagonal GMxC blocks are
         the per-class partial sums (the off-diagonal blocks are unused).
  - DMA PSUM -> DRAM; host sums diagonals across cores/groups, computes
    counts via bincount, and evaluates the tiny C-length loss.
"""

import sys

for _p in ("/opt/trn_rl_repo", "/root/.axon_site/_ro/trn_rl_repo"):
    if _p not in sys.path:
        sys.path.insert(0, _p)

from contextlib import ExitStack

import numpy as np

import concourse.bass as bass
from concourse import mybir, tile

# ---------------- problem constants (hardcoded per contract) ----------------
N_TOTAL = 1048576
C = 32
NCORES = 8
N_CORE = N_TOTAL // NCORES  # 131072
P = 128
PAGES = N_CORE // P         # 1024 points per partition
S_TILE = 64                 # pages (points) per partition per tile
TILES = PAGES // S_TILE     # 16
GM = 16                     # pages per matmul group == PSUM M dim
G = S_TILE // GM            # matmul groups per tile
NMM = GM * C                # 512 = rhs free dim per matmul

GAMMA = 0.3
LOSS_WEIGHT = 1.0
LG2 = 0.6931471805599453


# ---------------- custom DVE op: fused one-hot select ----------------------
def _register_onehot_gather():
    """out[p,s,n] = (n == in1[p,s,n]) ? in0[p,s,n] : 0, where the class index
    n is synthesized on-engine as Idx - PageIdx(0, N)."""
    from concourse import dve_ops
    from concourse.dve_spec import (
        C0,
        C1,
        Idx,
        PageIdx,
        Spec,
        Src0,
        Src1,
        Zero,
        eq,
        lower,
        select,
    )
    from concourse.dve_uop import DveOpSpec

    for op in dve_ops.OPS:
        if op.name == "ONEHOT_GATHER":
            return op

    def _ref(in0, in1, s0, s1, imm2):
        p = in0.shape[0]
        subdim = int(np.prod(in0.shape[1:-1]))
        n = in0.shape[-1]
        e = in0.reshape(p, subdim, n).astype(np.float32)
        lab = in1.reshape(p, subdim, n).astype(np.float32)
        s0v = float(s0.flat[0]) if isinstance(s0, np.ndarray) else float(s0)
        s1v = float(s1.flat[0]) if isinstance(s1, np.ndarray) else float(s1)
        idx = np.arange(subdim * n, dtype=np.float32).reshape(subdim, n)
        pg = s0v + np.arange(subdim, dtype=np.float32)[:, None] * s1v
        classidx = idx - pg
        return np.where(classidx == lab, e, np.float32(0.0)).reshape(in0.shape)

    spec = Spec(
        body=select(eq(Idx - PageIdx(C0, C1), Src1), Src0, Zero),
        reference=_ref,
    )
    shas = {}
    for ver in ("v3", "v4"):
        uops = lower(spec, ver=ver)
        shas[ver] = DveOpSpec(
            name="ONEHOT_GATHER", opcode=0, uops=uops, rd1_en=True
        ).sha(ver)
    op = dve_ops.DveOp("ONEHOT_GATHER", spec, subdim=True, uops_sha=shas)
    dve_ops.OPS.append(op)
    dve_ops.CUSTOM_DVE_SPECS[op.name] = op.spec
    dve_ops._SUB_OPCODE_FOR_NAME[op.name] = (
        max(dve_ops._SUB_OPCODE_FOR_NAME.values()) + 1
    )
    return op


ONEHOT_GATHER = _register_onehot_gather()


# ---------------- kernel builder -------------------------------------------
def build_nc(tiles: int = TILES):
    f32 = mybir.dt.float32
    f32r = mybir.dt.float32r
    nc = bass.Bass()
    cls_d = nc.dram_tensor("cls", [tiles, P, S_TILE * C], f32, kind="ExternalInput")
    lab_d = nc.dram_tensor("lab", [tiles, P, S_TILE], f32, kind="ExternalInput")
    out_d = nc.dram_tensor("out", [2, GM, NMM], f32, kind="ExternalOutput")

    with tile.TileContext(nc) as tc, ExitStack() as ctx:
        pool = ctx.enter_context(tc.tile_pool(name="work", bufs=3))
        spool = ctx.enter_context(tc.tile_pool(name="small", bufs=3))
        psum = ctx.enter_context(
            tc.tile_pool(name="psum", bufs=1, space=bass.MemorySpace.PSUM)
        )
        ps_s = psum.tile([GM, NMM], f32)
        ps_t = psum.tile([GM, NMM], f32)

        for t in range(tiles):
            x = pool.tile([P, S_TILE * C], f32, tag="x")
            nc.sync.dma_start(x[:], cls_d[t])
            lab = spool.tile([P, S_TILE], f32, tag="lab")
            nc.sync.dma_start(lab[:], lab_d[t])

            e = pool.tile([P, S_TILE * C], f32r, tag="e")
            nc.scalar.activation(e[:], x[:], mybir.ActivationFunctionType.Exp)
            e3 = e[:].rearrange("p (s n) -> p s n", n=C)

            den = spool.tile([P, S_TILE], f32, tag="den")
            nc.vector.tensor_reduce(
                den[:], e3, axis=mybir.AxisListType.X, op=mybir.AluOpType.add
            )
            rec = spool.tile([P, S_TILE], f32r, tag="rec")
            with nc.allow_low_precision(reason="f32r lhsT for PE matmul"):
                nc.vector.reciprocal(rec[:], den[:])

            # The custom-DVE InstISA has few sync-wait slots; absorb the
            # label-DMA dependency with a tiny stock op first (den already
            # absorbs the e dependency).
            scratch = spool.tile([P, 1], f32, tag="scratch")
            nc.vector.tensor_copy(scratch[:], lab[:, 0:1])

            m = pool.tile([P, S_TILE * C], f32r, tag="m")
            m3 = m[:].rearrange("p (s n) -> p s n", n=C)
            lab_b = lab[:].unsqueeze(2).broadcast_to([P, S_TILE, C])
            nc.vector._custom_dve(
                ONEHOT_GATHER, out=m3, in0=e3, in1=lab_b, s0=0.0, s1=float(C)
            )

            for g in range(G):
                first = t == 0 and g == 0
                last = t == tiles - 1 and g == G - 1
                rec_g = rec[:, g * GM : (g + 1) * GM]
                nc.tensor.matmul(
                    ps_s[:],
                    rec_g,
                    e[:, g * NMM : (g + 1) * NMM],
                    start=first,
                    stop=last,
                )
                nc.tensor.matmul(
                    ps_t[:],
                    rec_g,
                    m[:, g * NMM : (g + 1) * NMM],
                    start=first,
                    stop=last,
                )

        stage = pool.tile([GM, 2 * NMM], f32, tag="stage")
        nc.scalar.copy(stage[:, :NMM], ps_s[:])
        nc.scalar.copy(stage[:, NMM:], ps_t[:])
        nc.sync.dma_start(out_d[0], stage[:, :NMM])
        nc.sync.dma_start(out_d[1], stage[:, NMM:])
    return nc


def build_nc_v2(tiles: int = TILES):
    """v2: input is [tiles, P, S*(C+1)] where each page carries the 32 class
    scores plus the gathered true-class score. Device computes e = exp(all),
    D = sum of the first 32 per page, w = e[.., C]/D (per-point normalized
    true-class prob), and the S matmul. The per-class bucketing of w (ni)
    and counts happen on the host via bincount; no labels, no onehot, no
    T-matmul on device."""
    f32 = mybir.dt.float32
    f32r = mybir.dt.float32r
    nc = bass.Bass()
    cls_d = nc.dram_tensor("cls", [tiles, P, S_TILE * C], f32, kind="ExternalInput")
    g_d = nc.dram_tensor("gcol", [P, tiles * S_TILE], f32, kind="ExternalInput")
    out_d = nc.dram_tensor("out", [GM, NMM], f32, kind="ExternalOutput")
    w_d = nc.dram_tensor("wout", [P, tiles * S_TILE], f32, kind="ExternalOutput")

    pages = tiles * S_TILE
    with tile.TileContext(nc) as tc, ExitStack() as ctx:
        pool = ctx.enter_context(tc.tile_pool(name="work", bufs=6))
        spool = ctx.enter_context(tc.tile_pool(name="small", bufs=6))
        once = ctx.enter_context(tc.tile_pool(name="once", bufs=1))
        psum = ctx.enter_context(
            tc.tile_pool(name="psum", bufs=1, space=bass.MemorySpace.PSUM)
        )
        ps_s = psum.tile([GM, NMM], f32)

        gx_all = once.tile([P, pages], f32)
        eg_all = once.tile([P, pages], f32)
        w_all = once.tile([P, pages], f32)

        for t in range(tiles):
            x = pool.tile([P, S_TILE * C], f32, tag="x")
            nc.sync.dma_start(x[:], cls_d[t])

            if t == 0:
                # gathered true-class scores: one load + one exp, emitted
                # after the first cls DMA so they don't delay the pipeline
                nc.sync.dma_start(gx_all[:], g_d[:])
                nc.scalar.activation(
                    eg_all[:], gx_all[:], mybir.ActivationFunctionType.Exp
                )

            e = pool.tile([P, S_TILE * C], f32r, tag="e")
            nc.scalar.activation(e[:], x[:], mybir.ActivationFunctionType.Exp)
            e3 = e[:].rearrange("p (s n) -> p s n", n=C)

            den = spool.tile([P, S_TILE], f32, tag="den")
            nc.vector.tensor_reduce(
                den[:], e3, axis=mybir.AxisListType.X, op=mybir.AluOpType.add
            )
            rec0 = spool.tile([P, S_TILE], f32, tag="rec0")
            nc.vector.reciprocal_approx_fast(rec0[:], den[:])
            # f32 -> f32r rounding for the matmul lhsT on the Scalar engine
            # (ACT has slack; keeps the cast off the DVE critical path)
            rect = spool.tile([P, S_TILE], f32r, tag="rect")
            with nc.allow_low_precision(reason="f32r lhsT for PE matmul"):
                nc.scalar.mul(rect[:], rec0[:], 1.0)
            rec = rect[:]

            # per-point weight w = exp(g)/D on GPSIMD: keeps it off the DVE
            # critical path; GPSIMD only contends with 2-port DVE ops and the
            # steady-state DVE ops here are all single-port
            nc.gpsimd.tensor_tensor(
                w_all[:, t * S_TILE : (t + 1) * S_TILE],
                eg_all[:, t * S_TILE : (t + 1) * S_TILE],
                rec0[:],
                mybir.AluOpType.mult,
            )

            for g in range(G):
                first = t == 0 and g == 0
                last = t == tiles - 1 and g == G - 1
                nc.tensor.matmul(
                    ps_s[:],
                    rec[:, g * GM : (g + 1) * GM],
                    e[:, g * NMM : (g + 1) * NMM],
                    start=first,
                    stop=last,
                )

        nc.sync.dma_start(w_d[:], w_all[:])
        stage = pool.tile([GM, NMM], f32, tag="stage")
        nc.scalar.copy(stage[:], ps_s[:])
        nc.sync.dma_start(out_d[:], stage[:])
    return nc


def _finalize_for_hw(nc):
    """Lowerings required by the walrus compile path (not CoreSim)."""
    _split_multi_waits(nc)
    # Raw Bass does not run this pass; without it InstISA subclasses (the
    # custom DVE op) serialize with empty .instr -> "ISA wrong length".
    mybir.codegen_inst_isa_subclasses(nc)
    return nc


def _split_multi_waits(nc):
    """Walrus encodes exactly one sync-wait per ISA instruction; Tile can
    attach several. Hoist all-but-the-last wait onto single-wait NoOps
    inserted just before the instruction on the same engine (the sequencer
    executes them in order, so semantics are preserved)."""
    for fn in nc.m.functions:
        for blk in fn.blocks:
            new_list = []
            for ins in blk.instructions:
                si = ins.sync_info
                if si is not None and len(si.on_wait) > 1:
                    waits = list(si.on_wait)
                    for w in waits[:-1]:
                        nop = mybir.InstNoOp(
                            name=f"WS-{nc.next_id()}", ins=[], outs=[]
                        )
                        nop.engine = ins.engine
                        nop.sync_info = mybir.SyncInfo(on_wait=[w], on_update=[])
                        new_list.append(nop)
                    ins.sync_info = mybir.SyncInfo(
                        on_wait=[waits[-1]], on_update=list(si.on_update)
                    )
                new_list.append(ins)
            blk.instructions[:] = new_list


_NC_CACHE = {}


def _get_nc(tiles: int = TILES):
    if tiles not in _NC_CACHE:
        _NC_CACHE[tiles] = _finalize_for_hw(build_nc(tiles))
    return _NC_CACHE[tiles]


# ---------------- host-side driver (v2) -------------------------------------
def _prep_in_maps_v2(cls_score: np.ndarray, label: np.ndarray):
    cls_score = np.ascontiguousarray(cls_score, dtype=np.float32)
    lab = label.astype(np.int64)
    g = np.ascontiguousarray(cls_score[np.arange(cls_score.shape[0]), lab])
    in_maps = []
    for k in range(NCORES):
        sl = slice(k * N_CORE, (k + 1) * N_CORE)
        in_maps.append(
            {
                "cls": cls_score[sl].reshape(TILES, P, S_TILE * C),
                "gcol": np.ascontiguousarray(
                    g[sl].reshape(TILES, P, S_TILE).transpose(1, 0, 2)
                ).reshape(P, TILES * S_TILE),
            }
        )
    return in_maps


def _finalize_v2(outs, label: np.ndarray):
    lab = label.astype(np.int64)
    acc = np.zeros((GM, NMM), dtype=np.float64)
    w_parts = []
    for o in outs:
        acc += o["out"].astype(np.float64)
        w_parts.append(
            o["wout"].reshape(P, TILES, S_TILE).transpose(1, 0, 2).reshape(-1)
        )
    blocks = acc.reshape(GM, GM, C)
    s_c = np.zeros(C, dtype=np.float64)
    for mrow in range(GM):
        s_c += blocks[mrow, mrow]

    w_all = np.concatenate(w_parts).astype(np.float64)
    t_c = np.bincount(lab, weights=w_all, minlength=C)
    counts = np.bincount(lab, minlength=C).astype(np.float64)
    present = counts > 0
    ni = np.log(np.maximum(t_c, 1e-300))
    npr = np.log(np.maximum(s_c, 1e-300))
    log_ngt = np.log(np.maximum(counts, 1.0))
    log_dice = LG2 + ni - np.logaddexp(log_ngt, npr)
    neg_log_dice = np.where(present, -log_dice, 1.0)
    losses = np.where(present, np.power(np.maximum(neg_log_dice, 0.0), GAMMA), 0.0)
    n_present = present.sum()
    return np.float32(LOSS_WEIGHT * losses.sum() / n_present)


# ---------------- host-side driver (v1) -------------------------------------
def _prep_in_maps(cls_score: np.ndarray, label: np.ndarray):
    cls_score = np.ascontiguousarray(cls_score, dtype=np.float32)
    lab_f = label.astype(np.float32)
    in_maps = []
    for k in range(NCORES):
        sl = slice(k * N_CORE, (k + 1) * N_CORE)
        in_maps.append(
            {
                "cls": cls_score[sl].reshape(TILES, P, S_TILE * C),
                "lab": lab_f[sl].reshape(TILES, P, S_TILE),
            }
        )
    return in_maps


def _finalize(outs, label: np.ndarray):
    """outs: list (per core) of {"out": [2, GM, NMM]}; host reduction."""
    acc = np.zeros((2, GM, NMM), dtype=np.float64)
    for o in outs:
        acc += o["out"].astype(np.float64)
    blocks = acc.reshape(2, GM, GM, C)  # [2, m, j, c]; diagonal j == m
    s_c = np.zeros(C, dtype=np.float64)
    t_c = np.zeros(C, dtype=np.float64)
    for mrow in range(GM):
        s_c += blocks[0, mrow, mrow]
        t_c += blocks[1, mrow, mrow]

    counts = np.bincount(label.astype(np.int64), minlength=C).astype(np.float64)
    present = counts > 0
    ni = np.log(np.maximum(t_c, 1e-300))
    npr = np.log(np.maximum(s_c, 1e-300))
    log_ngt = np.log(np.maximum(counts, 1.0))
    log_dice = LG2 + ni - np.logaddexp(log_ngt, npr)
    neg_log_dice = np.where(present, -log_dice, 1.0)
    losses = np.where(present, np.power(np.maximum(neg_log_dice, 0.0), GAMMA), 0.0)
    n_present = present.sum()
    return np.float32(LOSS_WEIGHT * losses.sum() / n_present)


KERNEL_VERSION = 2


def _get_nc_v2(tiles: int = TILES):
    key = ("v2", tiles)
    if key not in _NC_CACHE:
        _NC_CACHE[key] = _finalize_for_hw(build_nc_v2(tiles))
    return _NC_CACHE[key]


def kernel(cls_score: np.ndarray, label: np.ndarray) -> np.ndarray:
    from concourse.bass_utils import run_bass_kernel_spmd

    cls_score = np.asarray(cls_score)
    label = np.asarray(label)
    assert cls_score.shape == (N_TOTAL, C), cls_score.shape
    if KERNEL_VERSION == 2:
        nc = _get_nc_v2()
        in_maps = _prep_in_maps_v2(cls_score, label)
        res = run_bass_kernel_spmd(nc, in_maps, core_ids=list(range(NCORES)))
        return _finalize_v2(res.results, label)
    nc = _get_nc()
    in_maps = _prep_in_maps(cls_score, label)
    res = run_bass_kernel_spmd(nc, in_maps, core_ids=list(range(NCORES)))
    return _finalize(res.results, label)


if __name__ == "__main__":
    rng = np.random.default_rng(0)
    x = rng.standard_normal((N_TOTAL, C), dtype=np.float32)
    lab = rng.integers(0, C, N_TOTAL).astype(np.int32)
    print("loss:", kernel(x, lab))



# revision 2
# speedup vs baseline: 1.1179x; 1.1179x over previous
"""ExpLog Dice loss kernel for Trainium2 (8 NeuronCores, SPMD data-parallel).

Math
----
reference computes, for cls_score [N, C] and integer labels [N]:
    log_probs = log_softmax(cls_score, axis=1)
    ni_c  = logsumexp_{n: label==c} log_probs[n, c]
    npr_c = logsumexp_n           log_probs[n, c]
    counts_c = #{n: label==c}
    ... tiny C-length final loss.

Since cls_score ~ N(0,1), exp(x) never overflows fp32, so logsumexps become
plain sums of probabilities:
    S_c = sum_n exp(x[n,c]) / D_n        (npr_c = log S_c)
    T_c = sum_{n:label=c} exp(x[n,c])/D_n (ni_c = log T_c)
    D_n = sum_c exp(x[n,c])

Device strategy (per core, N/8 = 131072 points, all-bf16 streaming):
  - host pre-casts cls_score to bf16 (halves HBM traffic; the loss is an
    average over 131k points/class so bf16 rounding noise vanishes)
  - layout: [128 partitions x pages x 32 classes], one point per page
  - ACT: e = exp(x) in bf16                              (the 1/elem pass)
  - DVE: D = within-page binary add-tree over the class axis (tensor_tensor
    at 2x bf16 rate; ~0.5 cyc/elem vs tensor_reduce's 1x)
  - DVE: r = 1/D (reciprocal_approx_fast) + bf16 cast
  - PE:  lhsT = r columns (16 pages/group), rhs = e; accumulate [16, 512]
         PSUM whose diagonal 16x32 blocks are the per-class sums S_c
  - outputs: two PSUM dumps (first/second half of tiles, so the first dump
    overlaps compute) + D per point; host computes w = exp(g)/D with the
    exact fp32 gathered true-class score, then bincounts T_c/counts and
    evaluates the tiny C-length loss.
"""

import sys

for _p in ("/opt/trn_rl_repo", "/root/.axon_site/_ro/trn_rl_repo"):
    if _p not in sys.path:
        sys.path.insert(0, _p)

from contextlib import ExitStack

import numpy as np

import concourse.bass as bass
from concourse import mybir, tile

# ---------------- problem constants (hardcoded per contract) ----------------
N_TOTAL = 1048576
C = 32
NCORES = 8
N_CORE = N_TOTAL // NCORES  # 131072
P = 128
PAGES = N_CORE // P         # 1024 points per partition
# variable tile sizes (pages): small first tiles shorten the DMA ramp
TILE_PAGES = [32, 96] + [128] * 7
assert sum(TILE_PAGES) == PAGES
GM = 16                     # pages per matmul group == PSUM M dim
NMM = GM * C                # 512 = rhs free dim per matmul
HALF_TILES = 5              # tiles 0..4 accumulate into psum A, rest into B

GAMMA = 0.3
LOSS_WEIGHT = 1.0
LG2 = 0.6931471805599453


# ---------------- kernel builder -------------------------------------------
def build_nc():
    f32 = mybir.dt.float32
    bf16 = mybir.dt.bfloat16
    nc = bass.Bass()
    cls_d = nc.dram_tensor(
        "cls", [P, PAGES * C], bf16, kind="ExternalInput"
    )
    out_d = nc.dram_tensor("out", [2, GM, NMM], f32, kind="ExternalOutput")
    den_d = nc.dram_tensor("den", [P, PAGES], f32, kind="ExternalOutput")

    ntiles = len(TILE_PAGES)
    starts = np.cumsum([0] + TILE_PAGES).tolist()

    with tile.TileContext(nc) as tc, ExitStack() as ctx:
        pool = ctx.enter_context(tc.tile_pool(name="work", bufs=3))
        spool = ctx.enter_context(tc.tile_pool(name="small", bufs=3))
        once = ctx.enter_context(tc.tile_pool(name="once", bufs=1))
        psum = ctx.enter_context(
            tc.tile_pool(name="psum", bufs=1, space=bass.MemorySpace.PSUM)
        )
        ps_a = psum.tile([GM, NMM], f32)
        ps_b = psum.tile([GM, NMM], f32)

        den_all = once.tile([P, PAGES], f32)
        stage = once.tile([GM, 2 * NMM], f32)

        for t in range(ntiles):
            s0, s = starts[t], TILE_PAGES[t]
            fd = s * C
            x = pool.tile([P, fd], bf16, tag="x")
            nc.sync.dma_start(x[:], cls_d[:, s0 * C : s0 * C + fd])

            e = pool.tile([P, fd], bf16, tag="e")
            nc.scalar.activation(e[:], x[:], mybir.ActivationFunctionType.Exp)
            e3 = e[:].rearrange("p (s n) -> p s n", n=C)

            # within-page add-tree over the class axis: 32 -> 16 -> ... -> 1.
            # tensor_tensor runs at 2x for bf16, so the whole tree costs
            # ~0.5 cyc/elem instead of tensor_reduce's 1 cyc/elem.
            h = e3
            for width in (16, 8, 4, 2):
                hn = pool.tile([P, s * width], bf16, tag=f"h{width}")
                hn3 = hn[:].rearrange("p (s n) -> p s n", n=width)
                nc.vector.tensor_add(hn3, h[:, :, 0:width], h[:, :, width : 2 * width])
                h = hn3
            den = den_all[:, s0 : s0 + s]
            nc.vector.tensor_add(den, h[:, :, 0], h[:, :, 1])

            rec = spool.tile([P, s], f32, tag="rec")
            nc.vector.reciprocal_approx_fast(rec[:], den)
            recb = spool.tile([P, s], bf16, tag="recb")
            with nc.allow_low_precision(reason="bf16 lhsT for PE matmul"):
                nc.vector.tensor_copy(recb[:], rec[:])

            # stream den out per tile so the output DMA overlaps compute
            nc.sync.dma_start(den_d[:, s0 : s0 + s], den)

            ps = ps_a if t < HALF_TILES else ps_b
            ngroups = s // GM
            for g in range(ngroups):
                first = g == 0 and t in (0, HALF_TILES)
                last = (t == HALF_TILES - 1 or t == ntiles - 1) and g == ngroups - 1
                nc.tensor.matmul(
                    ps[:],
                    recb[:, g * GM : (g + 1) * GM],
                    e[:, g * NMM : (g + 1) * NMM],
                    start=first,
                    stop=last,
                )
            if t == HALF_TILES - 1:
                # dump the first-half accumulator while the second half runs
                nc.vector.tensor_copy(stage[:, :NMM], ps_a[:])
                nc.sync.dma_start(out_d[0], stage[:, :NMM])

        nc.vector.tensor_copy(stage[:, NMM:], ps_b[:])
        nc.sync.dma_start(out_d[1], stage[:, NMM:])
    return nc


def _finalize_for_hw(nc):
    """Lowerings required by the walrus compile path (not CoreSim)."""
    _split_multi_waits(nc)
    mybir.codegen_inst_isa_subclasses(nc)
    return nc


def _split_multi_waits(nc):
    """Walrus encodes exactly one sync-wait per ISA instruction; Tile can
    attach several. Hoist all-but-the-last wait onto single-wait NoOps
    inserted just before the instruction on the same engine (the sequencer
    executes them in order, so semantics are preserved)."""
    for fn in nc.m.functions:
        for blk in fn.blocks:
            new_list = []
            for ins in blk.instructions:
                si = ins.sync_info
                if si is not None and len(si.on_wait) > 1:
                    waits = list(si.on_wait)
                    for w in waits[:-1]:
                        nop = mybir.InstNoOp(
                            name=f"WS-{nc.next_id()}", ins=[], outs=[]
                        )
                        nop.engine = ins.engine
                        nop.sync_info = mybir.SyncInfo(on_wait=[w], on_update=[])
                        new_list.append(nop)
                    ins.sync_info = mybir.SyncInfo(
                        on_wait=[waits[-1]], on_update=list(si.on_update)
                    )
                new_list.append(ins)
            blk.instructions[:] = new_list


_NC_CACHE = {}


def _get_nc():
    if "v3" not in _NC_CACHE:
        _NC_CACHE["v3"] = _finalize_for_hw(build_nc())
    return _NC_CACHE["v3"]


# ---------------- host-side driver ------------------------------------------
def _prep_in_maps(cls_score: np.ndarray, label: np.ndarray):
    import ml_dtypes

    cls_b = np.ascontiguousarray(cls_score, dtype=np.float32).astype(
        ml_dtypes.bfloat16
    )
    in_maps = []
    for k in range(NCORES):
        sl = slice(k * N_CORE, (k + 1) * N_CORE)
        # point n of the shard -> (partition p, page q): n = p*PAGES + q
        in_maps.append({"cls": cls_b[sl].reshape(P, PAGES * C)})
    return in_maps


def _finalize(outs, cls_score: np.ndarray, label: np.ndarray):
    lab = label.astype(np.int64)
    acc = np.zeros((GM, GM, C), dtype=np.float64)
    den_parts = []
    for o in outs:
        acc += o["out"].astype(np.float64).sum(axis=0).reshape(GM, GM, C)
        den_parts.append(o["den"].astype(np.float64).reshape(-1))
    s_c = np.zeros(C, dtype=np.float64)
    for mrow in range(GM):
        s_c += acc[mrow, mrow]

    # w_n = exp(g_n) / D_n with the exact fp32 true-class score g
    d_all = np.concatenate(den_parts)
    g = cls_score[np.arange(cls_score.shape[0]), lab].astype(np.float64)
    w_all = np.exp(g) / np.maximum(d_all, 1e-300)
    t_c = np.bincount(lab, weights=w_all, minlength=C)
    counts = np.bincount(lab, minlength=C).astype(np.float64)
    present = counts > 0
    ni = np.log(np.maximum(t_c, 1e-300))
    npr = np.log(np.maximum(s_c, 1e-300))
    log_ngt = np.log(np.maximum(counts, 1.0))
    log_dice = LG2 + ni - np.logaddexp(log_ngt, npr)
    neg_log_dice = np.where(present, -log_dice, 1.0)
    losses = np.where(present, np.power(np.maximum(neg_log_dice, 0.0), GAMMA), 0.0)
    n_present = present.sum()
    return np.float32(LOSS_WEIGHT * losses.sum() / n_present)


def kernel(cls_score: np.ndarray, label: np.ndarray) -> np.ndarray:
    from concourse.bass_utils import run_bass_kernel_spmd

    cls_score = np.asarray(cls_score)
    label = np.asarray(label)
    assert cls_score.shape == (N_TOTAL, C), cls_score.shape
    nc = _get_nc()
    in_maps = _prep_in_maps(cls_score, label)
    res = run_bass_kernel_spmd(nc, in_maps, core_ids=list(range(NCORES)))
    return _finalize(res.results, cls_score, label)


if __name__ == "__main__":
    rng = np.random.default_rng(0)
    x = rng.standard_normal((N_TOTAL, C), dtype=np.float32)
    lab = rng.integers(0, C, N_TOTAL).astype(np.int32)
    print("loss:", kernel(x, lab))


# revision 3
# speedup vs baseline: 1.1722x; 1.0486x over previous
"""ExpLog Dice loss kernel for Trainium2 (8 NeuronCores, SPMD data-parallel).

Math
----
reference computes, for cls_score [N, C] and integer labels [N]:
    log_probs = log_softmax(cls_score, axis=1)
    ni_c  = logsumexp_{n: label==c} log_probs[n, c]
    npr_c = logsumexp_n           log_probs[n, c]
    counts_c = #{n: label==c}
    ... tiny C-length final loss.

Since cls_score ~ N(0,1), exp(x) never overflows fp32, so logsumexps become
plain sums of probabilities:
    S_c = sum_n exp(x[n,c]) / D_n        (npr_c = log S_c)
    T_c = sum_{n:label=c} exp(x[n,c])/D_n (ni_c = log T_c)
    D_n = sum_c exp(x[n,c])

Device strategy (per core, N/8 = 131072 points, all-fp16 streaming):
  - host pre-casts cls_score to fp16 (halves HBM traffic; the loss is an
    average over 131k points/class so fp16 rounding noise vanishes)
  - layout: [128 partitions x pages x 32 classes], one point per page
  - ACT: e = exp(x) in fp16                              (the 1/elem pass)
  - DVE: D = within-page binary add-tree over the class axis (tensor_tensor
    at 2x 16-bit rate; ~0.5 cyc/elem vs tensor_reduce's 1x)
  - DVE: r = 1/D (reciprocal_approx_fast); GPSIMD casts r to fp16
  - PE:  lhsT = r columns (16 pages/group), rhs = e; groups alternate PE
         column quadrants (tile_position (0,0)/(0,32)) so each LDWEIGHTS
         overlaps the in-flight matmul on the other quadrant; accumulate
         [16, 512] PSUM regions whose diagonal 16x32 blocks are S_c partials
  - two accumulation rounds; the first round's PSUM dump overlaps round two
  - outputs: PSUM dumps + D per point; host computes w = exp(g)/D with the
    exact fp32 gathered true-class score, then bincounts T_c/counts and
    evaluates the tiny C-length loss.
"""

import sys

for _p in ("/opt/trn_rl_repo", "/root/.axon_site/_ro/trn_rl_repo"):
    if _p not in sys.path:
        sys.path.insert(0, _p)

from contextlib import ExitStack

import numpy as np

import concourse.bass as bass
from concourse import mybir, tile

# ---------------- problem constants (hardcoded per contract) ----------------
N_TOTAL = 1048576
C = 32
NCORES = 8
N_CORE = N_TOTAL // NCORES  # 131072
P = 128
PAGES = N_CORE // P         # 1024 points per partition
# variable tile sizes (pages): small first tiles shorten the DMA ramp, small
# last tiles shorten the serial tree->matmul tail
TILE_PAGES = [32, 96] + [128] * 6 + [64, 32, 32]
assert sum(TILE_PAGES) == PAGES
GM = 16                     # pages per matmul group == PSUM M dim
NMM = GM * C                # 512 = rhs free dim per matmul
ROUND_A_TILES = 6           # tiles 0..5 accumulate into round A, rest round B
QUADS = (0, 32)             # PE column quadrants to alternate between

GAMMA = 0.3
LOSS_WEIGHT = 1.0
LG2 = 0.6931471805599453


# ---------------- kernel builder -------------------------------------------
def build_nc():
    f32 = mybir.dt.float32
    f16 = mybir.dt.float16
    nc = bass.Bass()
    cls_d = nc.dram_tensor("cls", [P, PAGES * C], f16, kind="ExternalInput")
    # out[r, q] = round r, quadrant q [GM, NMM]
    out_d = nc.dram_tensor(
        "out", [2, len(QUADS), GM, NMM], f32, kind="ExternalOutput"
    )
    den_d = nc.dram_tensor("den", [P, PAGES], f32, kind="ExternalOutput")

    ntiles = len(TILE_PAGES)
    starts = np.cumsum([0] + TILE_PAGES).tolist()

    with tile.TileContext(nc) as tc, ExitStack() as ctx:
        pool = ctx.enter_context(tc.tile_pool(name="work", bufs=3))
        spool = ctx.enter_context(tc.tile_pool(name="small", bufs=3))
        once = ctx.enter_context(tc.tile_pool(name="once", bufs=1))
        psum = ctx.enter_context(
            tc.tile_pool(name="psum", bufs=1, space=bass.MemorySpace.PSUM)
        )
        ps = psum.tile([P, NMM], f32)

        den_all = once.tile([P, PAGES], f32)
        stage = once.tile([GM, 4 * NMM], f32)

        # (round, quad) -> [first_gidx, last_gidx] for start/stop flags
        gidx = 0
        bounds = {}
        for t in range(ntiles):
            rnd = 0 if t < ROUND_A_TILES else 1
            for g in range(TILE_PAGES[t] // GM):
                q = gidx % len(QUADS)
                key = (rnd, q)
                if key not in bounds:
                    bounds[key] = [gidx, gidx]
                bounds[key][1] = gidx
                gidx += 1

        gidx = 0
        for t in range(ntiles):
            s0, s = starts[t], TILE_PAGES[t]
            fd = s * C
            rnd = 0 if t < ROUND_A_TILES else 1
            x = pool.tile([P, fd], f16, tag="x")
            nc.sync.dma_start(x[:], cls_d[:, s0 * C : s0 * C + fd])

            e = pool.tile([P, fd], f16, tag="e")
            nc.scalar.activation(e[:], x[:], mybir.ActivationFunctionType.Exp)
            e3 = e[:].rearrange("p (s n) -> p s n", n=C)

            # within-page add-tree over the class axis: 32 -> 16 -> ... -> 1.
            # tensor_tensor runs at 2x for 16-bit dtypes, so the whole tree
            # costs ~0.5 cyc/elem instead of tensor_reduce's 1 cyc/elem.
            h = e3
            for width in (16, 8, 4, 2):
                hn = pool.tile([P, s * width], f16, tag=f"h{width}")
                hn3 = hn[:].rearrange("p (s n) -> p s n", n=width)
                nc.vector.tensor_add(hn3, h[:, :, 0:width], h[:, :, width : 2 * width])
                h = hn3
            den = den_all[:, s0 : s0 + s]
            nc.vector.tensor_add(den, h[:, :, 0], h[:, :, 1])

            rec = spool.tile([P, s], f32, tag="rec")
            nc.vector.reciprocal_approx_fast(rec[:], den)
            recb = spool.tile([P, s], f16, tag="recb")
            with nc.allow_low_precision(reason="fp16 lhsT for PE matmul"):
                nc.gpsimd.tensor_scalar_mul(recb[:], rec[:], 1.0)

            # stream den out per tile so the output DMA overlaps compute
            nc.sync.dma_start(den_d[:, s0 : s0 + s], den)

            for g in range(s // GM):
                q = gidx % len(QUADS)
                qb = QUADS[q]
                first = bounds[(rnd, q)][0] == gidx
                last = bounds[(rnd, q)][1] == gidx
                nc.tensor.matmul(
                    ps[qb : qb + GM, :],
                    recb[:, g * GM : (g + 1) * GM],
                    e[:, g * NMM : (g + 1) * NMM],
                    start=first,
                    stop=last,
                    tile_position=(0, qb),
                )
                gidx += 1

            if t == ROUND_A_TILES - 1:
                # dump round A while round B accumulates
                for q, qb in enumerate(QUADS):
                    dst = stage[:, q * NMM : (q + 1) * NMM]
                    nc.vector.tensor_copy(dst, ps[qb : qb + GM, :])
                    nc.sync.dma_start(out_d[0, q], dst)

        for q, qb in enumerate(QUADS):
            dst = stage[:, (2 + q) * NMM : (3 + q) * NMM]
            nc.vector.tensor_copy(dst, ps[qb : qb + GM, :])
            nc.sync.dma_start(out_d[1, q], dst)
    return nc


def _finalize_for_hw(nc):
    """Lowerings required by the walrus compile path (not CoreSim)."""
    _split_multi_waits(nc)
    mybir.codegen_inst_isa_subclasses(nc)
    return nc


def _split_multi_waits(nc):
    """Walrus encodes exactly one sync-wait per ISA instruction; Tile can
    attach several. Hoist all-but-the-last wait onto single-wait NoOps
    inserted just before the instruction on the same engine (the sequencer
    executes them in order, so semantics are preserved)."""
    for fn in nc.m.functions:
        for blk in fn.blocks:
            new_list = []
            for ins in blk.instructions:
                si = ins.sync_info
                if si is not None and len(si.on_wait) > 1:
                    waits = list(si.on_wait)
                    for w in waits[:-1]:
                        nop = mybir.InstNoOp(
                            name=f"WS-{nc.next_id()}", ins=[], outs=[]
                        )
                        nop.engine = ins.engine
                        nop.sync_info = mybir.SyncInfo(on_wait=[w], on_update=[])
                        new_list.append(nop)
                    ins.sync_info = mybir.SyncInfo(
                        on_wait=[waits[-1]], on_update=list(si.on_update)
                    )
                new_list.append(ins)
            blk.instructions[:] = new_list


_NC_CACHE = {}


def _get_nc():
    if "v4" not in _NC_CACHE:
        _NC_CACHE["v4"] = _finalize_for_hw(build_nc())
    return _NC_CACHE["v4"]


# ---------------- host-side driver ------------------------------------------
def _prep_in_maps(cls_score: np.ndarray, label: np.ndarray):
    cls_h = np.ascontiguousarray(cls_score, dtype=np.float32).astype(np.float16)
    in_maps = []
    for k in range(NCORES):
        sl = slice(k * N_CORE, (k + 1) * N_CORE)
        # point n of the shard -> (partition p, page q): n = p*PAGES + q
        in_maps.append({"cls": cls_h[sl].reshape(P, PAGES * C)})
    return in_maps


def _finalize(outs, cls_score: np.ndarray, label: np.ndarray):
    lab = label.astype(np.int64)
    acc = np.zeros((GM, GM, C), dtype=np.float64)
    den_parts = []
    for o in outs:
        acc += o["out"].astype(np.float64).sum(axis=(0, 1)).reshape(GM, GM, C)
        den_parts.append(o["den"].astype(np.float64).reshape(-1))
    s_c = np.zeros(C, dtype=np.float64)
    for mrow in range(GM):
        s_c += acc[mrow, mrow]

    # w_n = exp(g_n) / D_n with the exact fp32 true-class score g
    d_all = np.concatenate(den_parts)
    g = cls_score[np.arange(cls_score.shape[0]), lab].astype(np.float64)
    w_all = np.exp(g) / np.maximum(d_all, 1e-300)
    t_c = np.bincount(lab, weights=w_all, minlength=C)
    counts = np.bincount(lab, minlength=C).astype(np.float64)
    present = counts > 0
    ni = np.log(np.maximum(t_c, 1e-300))
    npr = np.log(np.maximum(s_c, 1e-300))
    log_ngt = np.log(np.maximum(counts, 1.0))
    log_dice = LG2 + ni - np.logaddexp(log_ngt, npr)
    neg_log_dice = np.where(present, -log_dice, 1.0)
    losses = np.where(present, np.power(np.maximum(neg_log_dice, 0.0), GAMMA), 0.0)
    n_present = present.sum()
    return np.float32(LOSS_WEIGHT * losses.sum() / n_present)


def kernel(cls_score: np.ndarray, label: np.ndarray) -> np.ndarray:
    from concourse.bass_utils import run_bass_kernel_spmd

    cls_score = np.asarray(cls_score)
    label = np.asarray(label)
    assert cls_score.shape == (N_TOTAL, C), cls_score.shape
    nc = _get_nc()
    in_maps = _prep_in_maps(cls_score, label)
    res = run_bass_kernel_spmd(nc, in_maps, core_ids=list(range(NCORES)))
    return _finalize(res.results, cls_score, label)


if __name__ == "__main__":
    rng = np.random.default_rng(0)
    x = rng.standard_normal((N_TOTAL, C), dtype=np.float32)
    lab = rng.integers(0, C, N_TOTAL).astype(np.int32)
    print("loss:", kernel(x, lab))


# revision 8
# speedup vs baseline: 1.3414x; 1.1443x over previous
"""ExpLog Dice loss kernel for Trainium2 (8 NeuronCores, SPMD data-parallel).

Math
----
reference computes, for cls_score [N, C] and integer labels [N]:
    log_probs = log_softmax(cls_score, axis=1)
    ni_c  = logsumexp_{n: label==c} log_probs[n, c]
    npr_c = logsumexp_n           log_probs[n, c]
    counts_c = #{n: label==c}
    ... tiny C-length final loss.

Since cls_score ~ N(0,1), exp(x) never overflows fp32, so logsumexps become
plain sums of probabilities:
    S_c = sum_n exp(x[n,c]) / D_n        (npr_c = log S_c)
    T_c = sum_{n:label=c} exp(x[n,c])/D_n (ni_c = log T_c)
    D_n = sum_c exp(x[n,c])

Device strategy (per core, N/8 = 131072 points, all-fp16 streaming):
  - host pre-casts cls_score to fp16 (halves HBM traffic; the loss is an
    average over 131k points/class so fp16 rounding noise vanishes)
  - layout: [128 partitions x pages x 32 classes], one point per page
  - ACT: e = exp(x) in fp16                              (the 1/elem pass)
  - DVE: D = within-page binary add-tree over the class axis (tensor_tensor
    at 2x 16-bit rate; ~0.5 cyc/elem vs tensor_reduce's 1x)
  - DVE: r = 1/D (reciprocal_approx_fast); GPSIMD casts r to fp16
  - PE:  lhsT = r columns (16 pages/group), rhs = e; groups alternate PE
         column quadrants (tile_position (0,0)/(0,32)) so each LDWEIGHTS
         overlaps the in-flight matmul on the other quadrant; accumulate
         [16, 512] PSUM regions whose diagonal 16x32 blocks are S_c partials
  - two accumulation rounds; the first round's PSUM dump overlaps round two
  - outputs: PSUM dumps + D per point; host computes w = exp(g)/D with the
    exact fp32 gathered true-class score, then bincounts T_c/counts and
    evaluates the tiny C-length loss.
"""

import sys

for _p in ("/opt/trn_rl_repo", "/root/.axon_site/_ro/trn_rl_repo"):
    if _p not in sys.path:
        sys.path.insert(0, _p)

from contextlib import ExitStack

import numpy as np

import concourse.bass as bass
from concourse import mybir, tile
from concourse.dve_ops import RECIP_APPROX_FAST_CONSTS, RECIPROCAL_APPROX_FAST

# ---------------- problem constants (hardcoded per contract) ----------------
N_TOTAL = 1048576
C = 32
NCORES = 8
N_CORE = N_TOTAL // NCORES  # 131072
P = 128
PAGES = N_CORE // P         # 1024 points per partition
# variable tile sizes (pages): small first tiles shorten the DMA ramp, small
# last tiles shorten the serial tree->matmul tail
TILE_PAGES = [32, 96] + [128] * 6 + [64, 32, 32]
assert sum(TILE_PAGES) == PAGES
GM = 16                     # pages per matmul group == PSUM M dim
NMM = GM * C                # 512 = rhs free dim per matmul
ROUND_A_TILES = 6           # tiles 0..5 accumulate into round A, rest round B
QUADS = (0, 32)             # PE column quadrants to alternate between
# den output batches: after tile t, DMA den_all pages [a, b)
_CUM = np.cumsum([0] + TILE_PAGES).tolist()
DEN_BATCH_AFTER = {4: (0, _CUM[5]), 8: (_CUM[5], _CUM[9]), 10: (_CUM[9], PAGES)}

GAMMA = 0.3
LOSS_WEIGHT = 1.0
LG2 = 0.6931471805599453


# ---------------- kernel builder -------------------------------------------
def build_nc():
    f32 = mybir.dt.float32
    f16 = mybir.dt.float16
    nc = bass.Bass()
    cls_d = nc.dram_tensor("cls", [P, PAGES * C], f16, kind="ExternalInput")
    # out[r, q] = round r, quadrant q [GM, NMM]
    out_d = nc.dram_tensor(
        "out", [2, len(QUADS), GM, NMM], f32, kind="ExternalOutput"
    )
    den_d = nc.dram_tensor("den", [P, PAGES], f32, kind="ExternalOutput")

    ntiles = len(TILE_PAGES)
    starts = np.cumsum([0] + TILE_PAGES).tolist()

    with tile.TileContext(nc) as tc, ExitStack() as ctx:
        pool = ctx.enter_context(tc.tile_pool(name="work", bufs=3))
        spool = ctx.enter_context(tc.tile_pool(name="small", bufs=3))
        once = ctx.enter_context(tc.tile_pool(name="once", bufs=1))
        psum = ctx.enter_context(
            tc.tile_pool(name="psum", bufs=1, space=bass.MemorySpace.PSUM)
        )
        ps = psum.tile([P, NMM], f32)

        den_all = once.tile([P, PAGES], f32)
        stage = once.tile([GM, 4 * NMM], f32)

        # tiny warm-up activation with no DMA dependency: hoists the one-time
        # ACT table load (~1.3us) into the preamble instead of serializing it
        # in front of the first real exp
        warm = once.tile([P, 1], f16)
        nc.vector.memset(warm[:], 0.0)
        nc.scalar.activation(warm[:], warm[:], mybir.ActivationFunctionType.Exp)

        # (round, quad) -> [first_gidx, last_gidx] for start/stop flags
        gidx = 0
        bounds = {}
        for t in range(ntiles):
            rnd = 0 if t < ROUND_A_TILES else 1
            for g in range(TILE_PAGES[t] // GM):
                q = gidx % len(QUADS)
                key = (rnd, q)
                if key not in bounds:
                    bounds[key] = [gidx, gidx]
                bounds[key][1] = gidx
                gidx += 1

        gidx = 0
        for t in range(ntiles):
            s0, s = starts[t], TILE_PAGES[t]
            fd = s * C
            rnd = 0 if t < ROUND_A_TILES else 1
            x = pool.tile([P, fd], f16, tag="x")
            nc.sync.dma_start(x[:], cls_d[:, s0 * C : s0 * C + fd])

            e = pool.tile([P, fd], f16, tag="e")
            nc.scalar.activation(e[:], x[:], mybir.ActivationFunctionType.Exp)
            e3 = e[:].rearrange("p (s n) -> p s n", n=C)

            # within-page add-tree over the class axis: 32 -> 16 -> ... -> 1.
            # tensor_tensor runs at 2x for 16-bit dtypes, so the whole tree
            # costs ~0.5 cyc/elem instead of tensor_reduce's 1 cyc/elem.
            h = e3
            for width in (16, 8, 4, 2):
                hn = pool.tile([P, s * width], f16, tag=f"h{width}")
                hn3 = hn[:].rearrange("p (s n) -> p s n", n=width)
                nc.vector.tensor_add(hn3, h[:, :, 0:width], h[:, :, width : 2 * width])
                h = hn3
            den = den_all[:, s0 : s0 + s]
            nc.vector.tensor_add(den, h[:, :, 0], h[:, :, 1])

            # approximate reciprocal straight to fp16 (the wrapper insists on
            # f32 out; the bit-trick seed only needs the f32 *input* layout)
            recb = spool.tile([P, s], f16, tag="recb")
            with nc.allow_low_precision(reason="fp16 lhsT for PE matmul"):
                c = RECIP_APPROX_FAST_CONSTS
                nc.vector._custom_dve(
                    RECIPROCAL_APPROX_FAST,
                    out=recb[:],
                    in0=den,
                    s0=c["s0"],
                    s1=c["s1"],
                    imm2=c["imm2"],
                )

            for g in range(s // GM):
                q = gidx % len(QUADS)
                qb = QUADS[q]
                first = bounds[(rnd, q)][0] == gidx
                last = bounds[(rnd, q)][1] == gidx
                nc.tensor.matmul(
                    ps[qb : qb + GM, :],
                    recb[:, g * GM : (g + 1) * GM],
                    e[:, g * NMM : (g + 1) * NMM],
                    start=first,
                    stop=last,
                    tile_position=(0, qb),
                )
                gidx += 1

            # batched den output on the idle GPSIMD (SWDGE) queue so the Sync
            # queue only ever issues input DMAs
            if t in DEN_BATCH_AFTER:
                a, b = DEN_BATCH_AFTER[t]
                nc.gpsimd.dma_start(den_d[:, a:b], den_all[:, a:b])

            if t == ROUND_A_TILES - 1:
                # dump round A while round B accumulates
                for q, qb in enumerate(QUADS):
                    dst = stage[:, q * NMM : (q + 1) * NMM]
                    nc.vector.tensor_copy(dst, ps[qb : qb + GM, :])
                    nc.gpsimd.dma_start(out_d[0, q], dst)

        for q, qb in enumerate(QUADS):
            dst = stage[:, (2 + q) * NMM : (3 + q) * NMM]
            nc.vector.tensor_copy(dst, ps[qb : qb + GM, :])
            nc.gpsimd.dma_start(out_d[1, q], dst)
    return nc


def _finalize_for_hw(nc):
    """Lowerings required by the walrus compile path (not CoreSim)."""
    _split_multi_waits(nc)
    mybir.codegen_inst_isa_subclasses(nc)
    return nc


def _split_multi_waits(nc):
    """Walrus encodes exactly one sync-wait per ISA instruction; Tile can
    attach several. Hoist all-but-the-last wait onto single-wait NoOps
    inserted just before the instruction on the same engine (the sequencer
    executes them in order, so semantics are preserved)."""
    for fn in nc.m.functions:
        for blk in fn.blocks:
            new_list = []
            for ins in blk.instructions:
                si = ins.sync_info
                if si is not None and len(si.on_wait) > 1:
                    waits = list(si.on_wait)
                    for w in waits[:-1]:
                        nop = mybir.InstNoOp(
                            name=f"WS-{nc.next_id()}", ins=[], outs=[]
                        )
                        nop.engine = ins.engine
                        nop.sync_info = mybir.SyncInfo(on_wait=[w], on_update=[])
                        new_list.append(nop)
                    ins.sync_info = mybir.SyncInfo(
                        on_wait=[waits[-1]], on_update=list(si.on_update)
                    )
                new_list.append(ins)
            blk.instructions[:] = new_list


_NC_CACHE = {}


def _get_nc():
    if "v4" not in _NC_CACHE:
        _NC_CACHE["v4"] = _finalize_for_hw(build_nc())
    return _NC_CACHE["v4"]


# ---------------- host-side driver ------------------------------------------
def _prep_in_maps(cls_score: np.ndarray, label: np.ndarray):
    cls_h = np.ascontiguousarray(cls_score, dtype=np.float32).astype(np.float16)
    in_maps = []
    for k in range(NCORES):
        sl = slice(k * N_CORE, (k + 1) * N_CORE)
        # point n of the shard -> (partition p, page q): n = p*PAGES + q
        in_maps.append({"cls": cls_h[sl].reshape(P, PAGES * C)})
    return in_maps


def _finalize(outs, cls_score: np.ndarray, label: np.ndarray):
    lab = label.astype(np.int64)
    acc = np.zeros((GM, GM, C), dtype=np.float64)
    den_parts = []
    for o in outs:
        acc += o["out"].astype(np.float64).sum(axis=(0, 1)).reshape(GM, GM, C)
        den_parts.append(o["den"].astype(np.float64).reshape(-1))
    s_c = np.zeros(C, dtype=np.float64)
    for mrow in range(GM):
        s_c += acc[mrow, mrow]

    # w_n = exp(g_n) / D_n with the exact fp32 true-class score g
    d_all = np.concatenate(den_parts)
    g = cls_score[np.arange(cls_score.shape[0]), lab].astype(np.float64)
    w_all = np.exp(g) / np.maximum(d_all, 1e-300)
    t_c = np.bincount(lab, weights=w_all, minlength=C)
    counts = np.bincount(lab, minlength=C).astype(np.float64)
    present = counts > 0
    ni = np.log(np.maximum(t_c, 1e-300))
    npr = np.log(np.maximum(s_c, 1e-300))
    log_ngt = np.log(np.maximum(counts, 1.0))
    log_dice = LG2 + ni - np.logaddexp(log_ngt, npr)
    neg_log_dice = np.where(present, -log_dice, 1.0)
    losses = np.where(present, np.power(np.maximum(neg_log_dice, 0.0), GAMMA), 0.0)
    n_present = present.sum()
    return np.float32(LOSS_WEIGHT * losses.sum() / n_present)


def kernel(cls_score: np.ndarray, label: np.ndarray) -> np.ndarray:
    from concourse.bass_utils import run_bass_kernel_spmd

    cls_score = np.asarray(cls_score)
    label = np.asarray(label)
    assert cls_score.shape == (N_TOTAL, C), cls_score.shape
    nc = _get_nc()
    in_maps = _prep_in_maps(cls_score, label)
    res = run_bass_kernel_spmd(nc, in_maps, core_ids=list(range(NCORES)))
    return _finalize(res.results, cls_score, label)


if __name__ == "__main__":
    rng = np.random.default_rng(0)
    x = rng.standard_normal((N_TOTAL, C), dtype=np.float32)
    lab = rng.integers(0, C, N_TOTAL).astype(np.int32)
    print("loss:", kernel(x, lab))
